# revision 19
# baseline (speedup 1.0000x reference)
"""Trainium2 Bass kernel for nn_GRU_15461882266204 (minGRU with causal conv gate).

Math (reference):
  w0 = x @ w_w.T ; z0 = x @ wz_w.T ; th = x @ wh_w.T          (S,H)
  z  = sigmoid(causal_conv4(z0, conv_w, segment-masked))
  a  = (1-z) * (1-start) ; b = z * th
  h_t = a_t * h_{t-1} + b_t                                    (scan over S)
  out = (h * silu(w0)) @ wo_w.T                                (S,D)

Strategy (v2): sequence-parallel over 8 NeuronCores (1024 positions each, all
5632 channels per core).  One fused phase A computes, per 128-channel m-tile:
the z / th / w0 projections (k-outer matmul groups sharing the xt moving
operand), the masked causal conv + sigmoid gates, the two hardware
tensor_tensor_scans (h_local, cumprod A), silu, and the bf16 products
g_loc = h_local*silu, gA = A*silu which stay SBUF-resident.  Cross-core scan
carry is exchanged in 11 chunked AllGathers of (A_end, h_end) summaries (4
m-tiles each) issued on the gpsimd queue as soon as each chunk's scans finish,
so collective latency overlaps the next chunk's matmuls.  The carry fixup
g = g_loc + carry*gA lands during phase A and writes final bf16 g tiles to
DRAM.  Phase D then runs 4 column passes of down-projection matmuls reading
pre-fixed g (2KB rows) + per-pass wo slices, sequence-sharded output (host
concatenates; no all-reduce).
"""
import sys

sys.path.insert(0, "/opt/trn_rl_repo")

import numpy as np

import concourse.bacc as bacc
import concourse.mybir as mybir
import concourse.tile as tile
from concourse.bass_utils import run_bass_kernel_spmd

try:
    import ml_dtypes

    BF16 = np.dtype(ml_dtypes.bfloat16)
except ImportError:  # pragma: no cover
    BF16 = None

F32 = mybir.dt.float32
MBF16 = mybir.dt.bfloat16
AL = mybir.AluOpType
ACTF = mybir.ActivationFunctionType

P = 128
CONV = 4
# 3 history columns are needed for the conv taps; pad to 4 (one dead leading
# column) to keep everything 4-aligned.
HIST = 4
CM = 4  # m-tiles per carry-exchange chunk


def build_gru_kernel(D, H, SC, NC):
    """Build the SPMD per-core program. SC = sequence chunk per core."""
    KT = D // P    # contraction k-tiles
    MT = H // P    # hidden m-tiles
    SCH = SC + HIST
    MPT = SC // P              # output row tiles (s on partitions)
    NB = D // 512              # down-proj column passes
    NCH = (MT + CM - 1) // CM  # carry chunks

    nc = bacc.Bacc(None, target_bir_lowering=False, debug=False)

    xt_in = nc.declare_dram_parameter("xt", [P, KT, SCH], MBF16, isOutput=False)
    wz_in = nc.declare_dram_parameter("wz", [MT, P, KT, P], MBF16, isOutput=False)
    wh_in = nc.declare_dram_parameter("wh", [MT, P, KT, P], MBF16, isOutput=False)
    w_in = nc.declare_dram_parameter("w", [MT, P, KT, P], MBF16, isOutput=False)
    wo_in = nc.declare_dram_parameter("wo", [NB, MT, P, 512], MBF16, isOutput=False)
    cw_in = nc.declare_dram_parameter("cw", [MT, P, CONV], F32, isOutput=False)
    u_in = nc.declare_dram_parameter("u", [P, SC + 2], MBF16, isOutput=False)
    sel_in = nc.declare_dram_parameter("sel", [P, NC], F32, isOutput=False)
    out_d = nc.declare_dram_parameter("out", [SC, D], F32, isOutput=True)

    with tile.TileContext(nc) as tc:
        with (
            tc.tile_pool(name="const", bufs=1) as cpool,
            tc.tile_pool(name="wts", bufs=3) as wpool,
            tc.tile_pool(name="work", bufs=2) as wk,
            tc.tile_pool(name="psum", bufs=8, space="PSUM") as pp,
            tc.tile_pool(name="dram", bufs=1, space="DRAM") as dp,
        ):
            # ---- resident tiles -------------------------------------------------
            # Ramp-critical ordering: the very first matmul needs wz[0] and xt
            # chunk 0 — issue those DMAs first so the fair-share DMA engines
            # complete them before the bulk (xt tail, u, sel) transfers.
            # xt is split column-wise at 516 (projection tile boundaries are
            # {4, 516, 1028}) and k-chunked, so m-tile 0's low-column matmul
            # group starts after only wz[0] + xt_lo (~2.6MB) of DMA.
            XCH = 4
            XKC = KT // XCH
            wz0_sb = wpool.tile([P, KT, P], MBF16, tag="wz", name="wz0_sb")
            nc.sync.dma_start(wz0_sb[:, 0:KT // 2, :], wz_in[0, :, 0:KT // 2, :])
            xtlo_sb, xthi_sb = [], []
            for j in range(XCH):
                xj = cpool.tile([P, XKC, 516], MBF16, tag=f"xtl{j}",
                                name=f"xtl{j}")
                nc.sync.dma_start(xj[:], xt_in[:, j * XKC:(j + 1) * XKC, 0:516])
                xtlo_sb.append(xj)
                if j == 0:
                    nc.sync.dma_start(wz0_sb[:, KT // 2:, :],
                                      wz_in[0, :, KT // 2:, :])
                elif j == 1:
                    wh0_sb = wpool.tile([P, KT, P], MBF16, tag="wh",
                                        name="wh0_sb")
                    nc.sync.dma_start(wh0_sb[:], wh_in[0])
                elif j == 2:
                    w0_sb = wpool.tile([P, KT, P], MBF16, tag="w", name="w0_sb")
                    nc.sync.dma_start(w0_sb[:], w_in[0])
            cw0_sb = wk.tile([P, CONV], F32, tag="cw", name="cw0_sb")
            nc.sync.dma_start(cw0_sb[:], cw_in[0])
            for j in range(XCH):
                xj = cpool.tile([P, XKC, SCH - 516], MBF16, tag=f"xth{j}",
                                name=f"xth{j}")
                nc.sync.dma_start(xj[:], xt_in[:, j * XKC:(j + 1) * XKC, 516:SCH])
                xthi_sb.append(xj)

            def xt_lo(k, lo, hi):  # cols [lo, hi) of xt, hi <= 516
                return xtlo_sb[k // XKC][:, k % XKC, lo:hi]

            def xt_hi(k, lo, hi):  # cols [lo, hi) of xt, lo >= 516
                return xthi_sb[k // XKC][:, k % XKC, lo - 516:hi - 516]

            u_sb = cpool.tile([P, SC + 2], MBF16, tag="u")
            nc.sync.dma_start(u_sb[:], u_in[:])
            sel_sb = cpool.tile([P, NC], F32, tag="sel")
            nc.sync.dma_start(sel_sb[:], sel_in[:])
            ones = cpool.tile([P, SC], MBF16, tag="ones")
            nc.any.memset(ones[:], 1.0)
            summA = cpool.tile([P, MT], F32, tag="summA")
            summH = cpool.tile([P, MT], F32, tag="summH")
            carry = cpool.tile([P, MT], F32, tag="carry")
            nc.vector.memset(carry[:], 0.0)

            # internal DRAM buffers
            g_d = dp.tile([MT, P, SC], MBF16)
            summ_d = [dp.tile([P, 2 * CM], F32, tag=f"summ{c}", name=f"summ{c}")
                      for c in range(NCH)]
            gath_d = [dp.tile([NC, P, 2 * CM], F32, addr_space="Shared",
                              tag=f"gath{c}", name=f"gath{c}")
                      for c in range(NCH)]

            # Warmup collective: absorb the first-rendezvous latency across
            # cores while the first m-tiles compute.
            warm_d = dp.tile([P, 2 * CM], F32, tag="warm", name="warm_d")
            warmg_d = dp.tile([NC, P, 2 * CM], F32, addr_space="Shared",
                              tag="warmg", name="warmg_d")
            warm_sb = cpool.tile([P, 2 * CM], F32, tag="warm_sb")
            nc.vector.memset(warm_sb[:], 0.0)
            nc.gpsimd.dma_start(warm_d[:], warm_sb[:])
            nc.gpsimd.collective_compute(
                "AllGather",
                AL.bypass,
                replica_groups=[list(range(NC))],
                ins=[warm_d.opt()],
                outs=[warmg_d.opt()],
            )

            gl_tiles = {}
            gA_tiles = {}
            pending = []  # chunks whose carry chain/fixup is deferred

            def process_chunk(c, m0, cm, gsum):
                """Emit carry chain + fixup for chunk c (gath already read)."""
                state = wk.tile([P, CM], F32, tag="cstate", name=f"cst{c}")
                tmp_c = wk.tile([P, CM], F32, tag="ctmp", name=f"ctm{c}")
                nc.vector.memset(state[:], 0.0)
                for r in range(NC):
                    if r > 0:
                        nc.vector.scalar_tensor_tensor(
                            carry[:, m0 : m0 + cm], state[:, :cm],
                            sel_sb[:, r : r + 1], carry[:, m0 : m0 + cm],
                            AL.mult, AL.add,
                        )
                    if r < NC - 1:
                        nc.vector.tensor_tensor(
                            tmp_c[:, :cm], state[:, :cm], gsum[r][:, 0:cm],
                            AL.mult
                        )
                        nc.vector.tensor_tensor(
                            state[:, :cm], tmp_c[:, :cm],
                            gsum[r][:, CM : CM + cm], AL.add
                        )
                for mm in range(m0, m0 + cm):
                    gfix = wk.tile([P, SC], MBF16, tag="gfix", name=f"gfx{mm}")
                    nc.vector.scalar_tensor_tensor(
                        gfix[:], gA_tiles[mm][:], carry[:, mm : mm + 1],
                        gl_tiles[mm][:], AL.mult, AL.add,
                    )
                    nc.gpsimd.dma_start(g_d[mm], gfix[:])
                    del gl_tiles[mm], gA_tiles[mm]

            scopeA = nc.named_scope("phaseA"); scopeA.__enter__()
            for m in range(MT):
                # flush deferred carry chunks once the collective has had ~2
                # m-tiles (~50us) to complete — keeps the DVE FIFO from
                # head-blocking on the gather.
                while pending and m >= pending[0][1] + pending[0][2] + 2:
                    process_chunk(*pending.pop(0))
                if m == 0:
                    cw_sb, wz_sb, wh_sb, w_sb = cw0_sb, wz0_sb, wh0_sb, w0_sb
                else:
                    cw_sb = wk.tile([P, CONV], F32, tag="cw")
                    nc.sync.dma_start(cw_sb[:], cw_in[m])
                    wz_sb = wpool.tile([P, KT, P], MBF16, tag="wz")
                    nc.sync.dma_start(wz_sb[:], wz_in[m])
                    wh_sb = wpool.tile([P, KT, P], MBF16, tag="wh")
                    nc.sync.dma_start(wh_sb[:], wh_in[m])
                    w_sb = wpool.tile([P, KT, P], MBF16, tag="w")
                    nc.sync.dma_start(w_sb[:], w_in[m])

                # low-column group: z [4,516) + z-tail [1,4), th/w0 [4,516)
                ps_z1 = pp.tile([P, 512], F32, tag="ps", name="ps_z1")
                ps_z3 = pp.tile([P, 512], F32, tag="ps", name="ps_z3")
                for k in range(KT):
                    nc.tensor.matmul(ps_z1[:, :512], wz_sb[:, k, :],
                                     xt_lo(k, 4, 516),
                                     start=(k == 0), stop=(k == KT - 1))
                    nc.tensor.matmul(ps_z3[:, :3], wz_sb[:, k, :],
                                     xt_lo(k, 1, 4),
                                     start=(k == 0), stop=(k == KT - 1))
                ps_hA = pp.tile([P, 512], F32, tag="ps", name="ps_hA")
                for k in range(KT):
                    nc.tensor.matmul(ps_hA[:, :512], wh_sb[:, k, :],
                                     xt_lo(k, 4, 516),
                                     start=(k == 0), stop=(k == KT - 1))
                ps_wA = pp.tile([P, 512], F32, tag="ps", name="ps_wA")
                for k in range(KT):
                    nc.tensor.matmul(ps_wA[:, :512], w_sb[:, k, :],
                                     xt_lo(k, 4, 516),
                                     start=(k == 0), stop=(k == KT - 1))
                # high-column group: cols [516, 1028)
                ps_z2 = pp.tile([P, 512], F32, tag="ps", name="ps_z2")
                for k in range(KT):
                    nc.tensor.matmul(ps_z2[:, :512], wz_sb[:, k, :],
                                     xt_hi(k, 516, 1028),
                                     start=(k == 0), stop=(k == KT - 1))
                ps_hB = pp.tile([P, 512], F32, tag="ps", name="ps_hB")
                for k in range(KT):
                    nc.tensor.matmul(ps_hB[:, :512], wh_sb[:, k, :],
                                     xt_hi(k, 516, 1028),
                                     start=(k == 0), stop=(k == KT - 1))
                ps_wB = pp.tile([P, 512], F32, tag="ps", name="ps_wB")
                for k in range(KT):
                    nc.tensor.matmul(ps_wB[:, :512], w_sb[:, k, :],
                                     xt_hi(k, 516, 1028),
                                     start=(k == 0), stop=(k == KT - 1))

                z_pre = wk.tile([P, SCH], MBF16, tag="zpre")
                nc.scalar.copy(z_pre[:, 1:4], ps_z3[:, :3])
                nc.scalar.copy(z_pre[:, 4:516], ps_z1[:, :512])
                nc.scalar.copy(z_pre[:, 516:SCH], ps_z2[:, :512])
                th_sb = wk.tile([P, SC], MBF16, tag="th", bufs=3)
                nc.scalar.copy(th_sb[:, 0:512], ps_hA[:, :512])
                nc.scalar.copy(th_sb[:, 512:1024], ps_hB[:, :512])
                silu_t = wk.tile([P, SC], MBF16, tag="silu")
                nc.scalar.activation(silu_t[:, 0:512], ps_wA[:, :512], ACTF.Silu)
                nc.scalar.activation(silu_t[:, 512:1024], ps_wB[:, :512], ACTF.Silu)

                # masked shifted taps: yk(t) = u(t) * y{k-1}(t-1), y0 = z_pre
                y1 = wk.tile([P, SC + 2], MBF16, tag="y1")
                nc.vector.tensor_tensor(
                    y1[:], u_sb[:, : SC + 2], z_pre[:, HIST - 3 : HIST - 3 + SC + 2],
                    AL.mult,
                )
                y2 = wk.tile([P, SC + 1], MBF16, tag="y2")
                nc.vector.tensor_tensor(
                    y2[:], u_sb[:, 1 : SC + 2], y1[:, : SC + 1], AL.mult
                )
                y3 = wk.tile([P, SC], MBF16, tag="y3")
                nc.vector.tensor_tensor(
                    y3[:], u_sb[:, 2 : SC + 2], y2[:, :SC], AL.mult
                )
                # conv accumulation: acc = z*cw3 + y1*cw2 + y2*cw1 + y3*cw0
                acc = wk.tile([P, SC], F32, tag="acc")
                nc.vector.tensor_scalar(
                    acc[:], z_pre[:, HIST:SCH], cw_sb[:, 3:4], None, AL.mult
                )
                nc.vector.scalar_tensor_tensor(
                    acc[:], y1[:, 2 : SC + 2], cw_sb[:, 2:3], acc[:], AL.mult, AL.add
                )
                nc.vector.scalar_tensor_tensor(
                    acc[:], y2[:, 1 : SC + 1], cw_sb[:, 1:2], acc[:], AL.mult, AL.add
                )
                nc.vector.scalar_tensor_tensor(
                    acc[:], y3[:, :SC], cw_sb[:, 0:1], acc[:], AL.mult, AL.add
                )

                z_t = wk.tile([P, SC], MBF16, tag="zt")
                nc.scalar.activation(z_t[:], acc[:], ACTF.Sigmoid)
                na = wk.tile([P, SC], MBF16, tag="na")
                nc.scalar.activation(na[:], acc[:], ACTF.Sigmoid, scale=-1.0)

                a_t = wk.tile([P, SC], MBF16, tag="a")
                nc.vector.tensor_tensor(a_t[:], na[:], u_sb[:, 2 : SC + 2], AL.mult)
                b_t = wk.tile([P, SC], MBF16, tag="b")
                nc.vector.tensor_tensor(b_t[:], z_t[:], th_sb[:], AL.mult)

                h_loc = wk.tile([P, SC], F32, tag="hl")
                nc.vector.tensor_tensor_scan(
                    h_loc[:], a_t[:], b_t[:], 0.0, AL.mult, AL.add
                )
                A_t = wk.tile([P, SC], F32, tag="A")
                nc.vector.tensor_tensor_scan(
                    A_t[:], a_t[:], ones[:], 1.0, AL.mult, AL.mult
                )

                nc.scalar.copy(summA[:, m : m + 1], A_t[:, SC - 1 : SC])
                nc.scalar.copy(summH[:, m : m + 1], h_loc[:, SC - 1 : SC])

                GBUFS = 2 * CM
                gl = wk.tile([P, SC], MBF16, tag="gl", bufs=GBUFS, name=f"gl{m}")
                nc.vector.tensor_tensor(gl[:], h_loc[:], silu_t[:], AL.mult)
                gA = wk.tile([P, SC], MBF16, tag="gA", bufs=GBUFS, name=f"gA{m}")
                nc.vector.tensor_tensor(gA[:], A_t[:], silu_t[:], AL.mult)
                gl_tiles[m] = gl
                gA_tiles[m] = gA

                # ---- chunk end: issue summary AllGather (carry deferred) --------
                if (m + 1) % CM == 0 or m == MT - 1:
                    c = m // CM
                    m0 = c * CM
                    cm = m + 1 - m0
                    nc.gpsimd.dma_start(summ_d[c][:, 0:cm], summA[:, m0 : m0 + cm])
                    nc.gpsimd.dma_start(summ_d[c][:, CM : CM + cm],
                                        summH[:, m0 : m0 + cm])
                    nc.gpsimd.collective_compute(
                        "AllGather",
                        AL.bypass,
                        replica_groups=[list(range(NC))],
                        ins=[summ_d[c].opt()],
                        outs=[gath_d[c].opt()],
                    )
                    gsum = []
                    for r in range(NC):
                        gs = wk.tile([P, 2 * CM], F32, tag=f"gsum{r}",
                                     name=f"gsum{r}_{c}")
                        nc.gpsimd.dma_start(gs[:], gath_d[c][r])
                        gsum.append(gs)
                    pending.append((c, m0, cm, gsum))
            while pending:
                process_chunk(*pending.pop(0))
            scopeA.__exit__(None, None, None)

            # ---- phase D: down-projection out = g.T @ wo -----------------------
            scopeD = nc.named_scope("phaseD"); scopeD.__enter__()
            for nb in range(NB):
                ps_o = [pp.tile([P, 512], F32, tag="ps", name=f"pso{i}")
                        for i in range(MPT)]
                for m in range(MT):
                    g_rd = wk.tile([P, SC], MBF16, tag="g_rd", bufs=4)
                    nc.sync.dma_start(g_rd[:], g_d[m])
                    wo_rd = wk.tile([P, 512], MBF16, tag="wo_rd", bufs=4)
                    nc.sync.dma_start(wo_rd[:], wo_in[nb, m])
                    for mb in range(MPT):
                        nc.tensor.matmul(
                            ps_o[mb][:, :512],
                            g_rd[:, mb * P : (mb + 1) * P],
                            wo_rd[:],
                            start=(m == 0),
                            stop=(m == MT - 1),
                        )
                for mb in range(MPT):
                    o_sb = wk.tile([P, 512], F32, tag="o_sb", bufs=8)
                    if mb % 2 == 0:
                        nc.vector.tensor_copy(o_sb[:], ps_o[mb][:, :512])
                    else:
                        nc.scalar.copy(o_sb[:], ps_o[mb][:, :512])
                    # out writes ride the scalar hwdge queue so they never
                    # head-block the next pass's g/wo prefetch on sync; the
                    # final pass splits across both queues to shorten the tail.
                    dma_eng = nc.scalar if (nb < NB - 1 or mb % 2) else nc.sync
                    dma_eng.dma_start(
                        out_d[mb * P : (mb + 1) * P, nb * 512 : (nb + 1) * 512],
                        o_sb[:],
                    )
            scopeD.__exit__(None, None, None)
    nc.compile()
    return nc


def _prep_inputs(x, cu_seqlens, w_w, wz_w, wh_w, wo_w, conv_w, NC):
    """Host-side sharding + layout prep. Returns in_maps list."""
    S, D = x.shape[1], x.shape[2]
    H = w_w.shape[0]
    SC = S // NC
    KT, MT = D // P, H // P
    NB = D // 512

    xT = np.ascontiguousarray(x[0].T.astype(np.float32))  # (D, S)
    xt_full = np.zeros((D, S + HIST), np.float32)
    xt_full[:, HIST:] = xT

    start = np.zeros(S, np.float32)
    for v in np.asarray(cu_seqlens[:-1]):
        v = int(v)
        if 0 <= v < S:
            start[v] = 1.0
    u = 1.0 - start
    u_full = np.ones(S + 2, np.float32)
    u_full[2:] = u  # index t+2 <-> position t

    def wprep(wm):  # (H, D) -> (MT, P, KT, P) with [m,p,k,j] = w[m*P+j, k*P+p]
        return np.ascontiguousarray(
            wm.astype(np.float32).reshape(MT, P, KT, P).transpose(0, 3, 2, 1)
        ).astype(BF16)

    wz_t, wh_t, w_t = wprep(wz_w), wprep(wh_w), wprep(w_w)
    # wo: [NB, MT, P, 512] with [nb,m,p,j] = wo[nb*512+j, m*128+p]
    wo_t = np.ascontiguousarray(
        wo_w.T.astype(np.float32).reshape(MT, P, NB, 512).transpose(2, 0, 1, 3)
    ).astype(BF16)
    cw_t = np.ascontiguousarray(conv_w.astype(np.float32).reshape(MT, P, CONV))

    in_maps = []
    for c in range(NC):
        s0 = c * SC
        xt_c = np.ascontiguousarray(
            xt_full[:, s0 : s0 + SC + HIST]
            .reshape(KT, P, SC + HIST)
            .transpose(1, 0, 2)
        ).astype(BF16)
        u_c = np.ascontiguousarray(
            np.broadcast_to(u_full[s0 : s0 + SC + 2], (P, SC + 2))
        ).astype(BF16)
        sel_c = np.zeros((P, NC), np.float32)
        sel_c[:, c] = 1.0
        in_maps.append(
            {
                "xt": xt_c,
                "wz": wz_t,
                "wh": wh_t,
                "w": w_t,
                "wo": wo_t,
                "cw": cw_t,
                "u": u_c,
                "sel": sel_c,
            }
        )
    return in_maps


_NC_CACHE = {}


def run_gru(x, cu_seqlens, w_w, wz_w, wh_w, wo_w, conv_w, NC=8, trace=False):
    S, D = x.shape[1], x.shape[2]
    H = w_w.shape[0]
    SC = S // NC
    key = (D, H, SC, NC)
    if key not in _NC_CACHE:
        _NC_CACHE[key] = build_gru_kernel(D, H, SC, NC)
    nc = _NC_CACHE[key]
    in_maps = _prep_inputs(x, cu_seqlens, w_w, wz_w, wh_w, wo_w, conv_w, NC)
    res = run_bass_kernel_spmd(nc, in_maps, list(range(NC)), trace=trace)
    out = np.concatenate([res.results[c]["out"] for c in range(NC)], axis=0)
    return out.reshape(1, S, D).astype(np.float32), res


def kernel(**inputs):
    out, _ = run_gru(
        inputs["x"],
        inputs["cu_seqlens"],
        inputs["w_w"],
        inputs["wz_w"],
        inputs["wh_w"],
        inputs["wo_w"],
        inputs["conv_w"],
        NC=8,
    )
    return out


# revision 21
# speedup vs baseline: 1.0055x; 1.0055x over previous
"""Trainium2 Bass kernel for nn_GRU_15461882266204 (minGRU with causal conv gate).

Math (reference):
  w0 = x @ w_w.T ; z0 = x @ wz_w.T ; th = x @ wh_w.T          (S,H)
  z  = sigmoid(causal_conv4(z0, conv_w, segment-masked))
  a  = (1-z) * (1-start) ; b = z * th
  h_t = a_t * h_{t-1} + b_t                                    (scan over S)
  out = (h * silu(w0)) @ wo_w.T                                (S,D)

Strategy (v2): sequence-parallel over 8 NeuronCores (1024 positions each, all
5632 channels per core).  One fused phase A computes, per 128-channel m-tile:
the z / th / w0 projections (k-outer matmul groups sharing the xt moving
operand), the masked causal conv + sigmoid gates, the two hardware
tensor_tensor_scans (h_local, cumprod A), silu, and the bf16 products
g_loc = h_local*silu, gA = A*silu which stay SBUF-resident.  Cross-core scan
carry is exchanged in 11 chunked AllGathers of (A_end, h_end) summaries (4
m-tiles each) issued on the gpsimd queue as soon as each chunk's scans finish,
so collective latency overlaps the next chunk's matmuls.  The carry fixup
g = g_loc + carry*gA lands during phase A and writes final bf16 g tiles to
DRAM.  Phase D then runs 4 column passes of down-projection matmuls reading
pre-fixed g (2KB rows) + per-pass wo slices, sequence-sharded output (host
concatenates; no all-reduce).
"""
import sys

sys.path.insert(0, "/opt/trn_rl_repo")

import numpy as np

import concourse.bacc as bacc
import concourse.mybir as mybir
import concourse.tile as tile
from concourse.bass_utils import run_bass_kernel_spmd

try:
    import ml_dtypes

    BF16 = np.dtype(ml_dtypes.bfloat16)
except ImportError:  # pragma: no cover
    BF16 = None

F32 = mybir.dt.float32
MBF16 = mybir.dt.bfloat16
AL = mybir.AluOpType
ACTF = mybir.ActivationFunctionType

P = 128
CONV = 4
# 3 history columns are needed for the conv taps; pad to 4 (one dead leading
# column) to keep everything 4-aligned.
HIST = 4
CM = 4  # m-tiles per carry-exchange chunk


def build_gru_kernel(D, H, SC, NC):
    """Build the SPMD per-core program. SC = sequence chunk per core."""
    KT = D // P    # contraction k-tiles
    MT = H // P    # hidden m-tiles
    SCH = SC + HIST
    MPT = SC // P              # output row tiles (s on partitions)
    NB = D // 512              # down-proj column passes
    NCH = (MT + CM - 1) // CM  # carry chunks

    nc = bacc.Bacc(None, target_bir_lowering=False, debug=False)

    xt_in = nc.declare_dram_parameter("xt", [P, KT, SCH], MBF16, isOutput=False)
    wz_in = nc.declare_dram_parameter("wz", [MT, P, KT, P], MBF16, isOutput=False)
    wh_in = nc.declare_dram_parameter("wh", [MT, P, KT, P], MBF16, isOutput=False)
    w_in = nc.declare_dram_parameter("w", [MT, P, KT, P], MBF16, isOutput=False)
    wo_in = nc.declare_dram_parameter("wo", [NB, MT, P, 512], MBF16, isOutput=False)
    cw_in = nc.declare_dram_parameter("cw", [MT, P, CONV], F32, isOutput=False)
    u_in = nc.declare_dram_parameter("u", [P, SC + 2], MBF16, isOutput=False)
    sel_in = nc.declare_dram_parameter("sel", [P, NC], F32, isOutput=False)
    out_d = nc.declare_dram_parameter("out", [SC, D], F32, isOutput=True)

    with tile.TileContext(nc) as tc:
        with (
            tc.tile_pool(name="const", bufs=1) as cpool,
            tc.tile_pool(name="wts", bufs=3) as wpool,
            tc.tile_pool(name="work", bufs=2) as wk,
            tc.tile_pool(name="psum", bufs=8, space="PSUM") as pp,
            tc.tile_pool(name="dram", bufs=1, space="DRAM") as dp,
        ):
            # ---- resident tiles -------------------------------------------------
            # Ramp-critical ordering: the very first matmul needs wz[0] and xt
            # chunk 0 — issue those DMAs first so the fair-share DMA engines
            # complete them before the bulk (xt tail, u, sel) transfers.
            XCH = 4
            wz0_sb = wpool.tile([P, KT, P], MBF16, tag="wz", name="wz0_sb")
            nc.sync.dma_start(wz0_sb[:, 0:KT // 2, :], wz_in[0, :, 0:KT // 2, :])
            nc.sync.dma_start(wz0_sb[:, KT // 2:, :], wz_in[0, :, KT // 2:, :])
            xt_sb = []
            xj = cpool.tile([P, KT // XCH, SCH], MBF16, tag="xt0", name="xt0")
            nc.sync.dma_start(xj[:], xt_in[:, 0:KT // XCH, :])
            xt_sb.append(xj)
            wh0_sb = wpool.tile([P, KT, P], MBF16, tag="wh", name="wh0_sb")
            nc.sync.dma_start(wh0_sb[:], wh_in[0])
            for j in range(1, XCH):
                xj = cpool.tile([P, KT // XCH, SCH], MBF16, tag=f"xt{j}",
                                name=f"xt{j}")
                nc.sync.dma_start(xj[:], xt_in[:, j * (KT // XCH):(j + 1) * (KT // XCH), :])
                xt_sb.append(xj)
            w0_sb = wpool.tile([P, KT, P], MBF16, tag="w", name="w0_sb")
            nc.sync.dma_start(w0_sb[:], w_in[0])
            cw0_sb = wk.tile([P, CONV], F32, tag="cw", name="cw0_sb")
            nc.sync.dma_start(cw0_sb[:], cw_in[0])

            def xt_slice(k, lo, hi):
                j = k // (KT // XCH)
                return xt_sb[j][:, k % (KT // XCH), lo:hi]

            u_sb = cpool.tile([P, SC + 2], MBF16, tag="u")
            nc.sync.dma_start(u_sb[:], u_in[:])
            sel_sb = cpool.tile([P, NC], F32, tag="sel")
            nc.sync.dma_start(sel_sb[:], sel_in[:])
            ones = cpool.tile([P, SC], MBF16, tag="ones")
            nc.any.memset(ones[:], 1.0)
            summA = cpool.tile([P, MT], F32, tag="summA")
            summH = cpool.tile([P, MT], F32, tag="summH")
            carry = cpool.tile([P, MT], F32, tag="carry")
            nc.vector.memset(carry[:], 0.0)

            # internal DRAM buffers
            g_d = dp.tile([MT, P, SC], MBF16)
            summ_d = [dp.tile([P, 2 * CM], F32, tag=f"summ{c}", name=f"summ{c}")
                      for c in range(NCH)]
            gath_d = [dp.tile([NC, P, 2 * CM], F32, addr_space="Shared",
                              tag=f"gath{c}", name=f"gath{c}")
                      for c in range(NCH)]

            # Warmup collective: absorb the first-rendezvous latency across
            # cores while the first m-tiles compute.
            warm_d = dp.tile([P, 2 * CM], F32, tag="warm", name="warm_d")
            warmg_d = dp.tile([NC, P, 2 * CM], F32, addr_space="Shared",
                              tag="warmg", name="warmg_d")
            warm_sb = cpool.tile([P, 2 * CM], F32, tag="warm_sb")
            nc.vector.memset(warm_sb[:], 0.0)
            nc.gpsimd.dma_start(warm_d[:], warm_sb[:])
            nc.gpsimd.collective_compute(
                "AllGather",
                AL.bypass,
                replica_groups=[list(range(NC))],
                ins=[warm_d.opt()],
                outs=[warmg_d.opt()],
            )

            gl_tiles = {}
            gA_tiles = {}
            pending = []  # chunks whose carry chain/fixup is deferred

            def process_chunk(c, m0, cm, gsum):
                """Emit carry chain + fixup for chunk c (gath already read)."""
                state = wk.tile([P, CM], F32, tag="cstate", name=f"cst{c}")
                tmp_c = wk.tile([P, CM], F32, tag="ctmp", name=f"ctm{c}")
                nc.vector.memset(state[:], 0.0)
                for r in range(NC):
                    if r > 0:
                        nc.vector.scalar_tensor_tensor(
                            carry[:, m0 : m0 + cm], state[:, :cm],
                            sel_sb[:, r : r + 1], carry[:, m0 : m0 + cm],
                            AL.mult, AL.add,
                        )
                    if r < NC - 1:
                        nc.vector.tensor_tensor(
                            tmp_c[:, :cm], state[:, :cm], gsum[r][:, 0:cm],
                            AL.mult
                        )
                        nc.vector.tensor_tensor(
                            state[:, :cm], tmp_c[:, :cm],
                            gsum[r][:, CM : CM + cm], AL.add
                        )
                for mm in range(m0, m0 + cm):
                    gfix = wk.tile([P, SC], MBF16, tag="gfix", name=f"gfx{mm}")
                    nc.vector.scalar_tensor_tensor(
                        gfix[:], gA_tiles[mm][:], carry[:, mm : mm + 1],
                        gl_tiles[mm][:], AL.mult, AL.add,
                    )
                    nc.gpsimd.dma_start(g_d[mm], gfix[:])
                    del gl_tiles[mm], gA_tiles[mm]

            scopeA = nc.named_scope("phaseA"); scopeA.__enter__()
            for m in range(MT):
                # flush deferred carry chunks once the collective has had ~2
                # m-tiles (~50us) to complete — keeps the DVE FIFO from
                # head-blocking on the gather.
                while pending and m >= pending[0][1] + pending[0][2] + 2:
                    process_chunk(*pending.pop(0))
                if m == 0:
                    cw_sb, wz_sb, wh_sb, w_sb = cw0_sb, wz0_sb, wh0_sb, w0_sb
                else:
                    cw_sb = wk.tile([P, CONV], F32, tag="cw")
                    nc.sync.dma_start(cw_sb[:], cw_in[m])
                    wz_sb = wpool.tile([P, KT, P], MBF16, tag="wz")
                    nc.sync.dma_start(wz_sb[:], wz_in[m])
                    wh_sb = wpool.tile([P, KT, P], MBF16, tag="wh")
                    nc.sync.dma_start(wh_sb[:], wh_in[m])
                    w_sb = wpool.tile([P, KT, P], MBF16, tag="w")
                    nc.sync.dma_start(w_sb[:], w_in[m])

                # z_pre = wz_m.T @ x over SC+4 cols (history included), k-outer
                ps_z = [pp.tile([P, 512], F32, tag="ps", name=f"psz{i}")
                        for i in range(3)]
                for k in range(KT):
                    nc.tensor.matmul(ps_z[0][:, :512], wz_sb[:, k, :],
                                     xt_slice(k, 0, 512),
                                     start=(k == 0), stop=(k == KT - 1))
                    nc.tensor.matmul(ps_z[1][:, :512], wz_sb[:, k, :],
                                     xt_slice(k, 512, 1024),
                                     start=(k == 0), stop=(k == KT - 1))
                    nc.tensor.matmul(ps_z[2][:, :SCH - 1024], wz_sb[:, k, :],
                                     xt_slice(k, 1024, SCH),
                                     start=(k == 0), stop=(k == KT - 1))
                z_pre = wk.tile([P, SCH], MBF16, tag="zpre")
                nc.scalar.copy(z_pre[:, 0:512], ps_z[0][:, :512])
                nc.scalar.copy(z_pre[:, 512:1024], ps_z[1][:, :512])
                nc.scalar.copy(z_pre[:, 1024:SCH], ps_z[2][:, :SCH - 1024])

                # th matmuls (positions [0, SC) = cols [HIST, SCH))
                ps_h = [pp.tile([P, 512], F32, tag="ps", name=f"psh{i}")
                        for i in range(2)]
                for k in range(KT):
                    for i in range(2):
                        nc.tensor.matmul(
                            ps_h[i][:, :512], wh_sb[:, k, :],
                            xt_slice(k, HIST + i * 512, HIST + (i + 1) * 512),
                            start=(k == 0), stop=(k == KT - 1))
                th_sb = wk.tile([P, SC], MBF16, tag="th", bufs=3)
                nc.scalar.copy(th_sb[:, 0:512], ps_h[0][:, :512])
                nc.scalar.copy(th_sb[:, 512:1024], ps_h[1][:, :512])

                # w0 matmuls + silu
                ps_w = [pp.tile([P, 512], F32, tag="ps", name=f"psw{i}")
                        for i in range(2)]
                for k in range(KT):
                    for i in range(2):
                        nc.tensor.matmul(
                            ps_w[i][:, :512], w_sb[:, k, :],
                            xt_slice(k, HIST + i * 512, HIST + (i + 1) * 512),
                            start=(k == 0), stop=(k == KT - 1))
                silu_t = wk.tile([P, SC], MBF16, tag="silu")
                nc.scalar.activation(silu_t[:, 0:512], ps_w[0][:, :512], ACTF.Silu)
                nc.scalar.activation(silu_t[:, 512:1024], ps_w[1][:, :512], ACTF.Silu)

                # masked shifted taps: yk(t) = u(t) * y{k-1}(t-1), y0 = z_pre
                y1 = wk.tile([P, SC + 2], MBF16, tag="y1")
                nc.vector.tensor_tensor(
                    y1[:], u_sb[:, : SC + 2], z_pre[:, HIST - 3 : HIST - 3 + SC + 2],
                    AL.mult,
                )
                y2 = wk.tile([P, SC + 1], MBF16, tag="y2")
                nc.vector.tensor_tensor(
                    y2[:], u_sb[:, 1 : SC + 2], y1[:, : SC + 1], AL.mult
                )
                y3 = wk.tile([P, SC], MBF16, tag="y3")
                nc.vector.tensor_tensor(
                    y3[:], u_sb[:, 2 : SC + 2], y2[:, :SC], AL.mult
                )
                # conv accumulation: acc = z*cw3 + y1*cw2 + y2*cw1 + y3*cw0
                acc = wk.tile([P, SC], F32, tag="acc")
                nc.vector.tensor_scalar(
                    acc[:], z_pre[:, HIST:SCH], cw_sb[:, 3:4], None, AL.mult
                )
                nc.vector.scalar_tensor_tensor(
                    acc[:], y1[:, 2 : SC + 2], cw_sb[:, 2:3], acc[:], AL.mult, AL.add
                )
                nc.vector.scalar_tensor_tensor(
                    acc[:], y2[:, 1 : SC + 1], cw_sb[:, 1:2], acc[:], AL.mult, AL.add
                )
                nc.vector.scalar_tensor_tensor(
                    acc[:], y3[:, :SC], cw_sb[:, 0:1], acc[:], AL.mult, AL.add
                )

                z_t = wk.tile([P, SC], MBF16, tag="zt")
                nc.scalar.activation(z_t[:], acc[:], ACTF.Sigmoid)
                na = wk.tile([P, SC], MBF16, tag="na")
                nc.scalar.activation(na[:], acc[:], ACTF.Sigmoid, scale=-1.0)

                a_t = wk.tile([P, SC], MBF16, tag="a")
                nc.vector.tensor_tensor(a_t[:], na[:], u_sb[:, 2 : SC + 2], AL.mult)
                b_t = wk.tile([P, SC], MBF16, tag="b")
                nc.vector.tensor_tensor(b_t[:], z_t[:], th_sb[:], AL.mult)

                h_loc = wk.tile([P, SC], F32, tag="hl")
                nc.vector.tensor_tensor_scan(
                    h_loc[:], a_t[:], b_t[:], 0.0, AL.mult, AL.add
                )
                A_t = wk.tile([P, SC], F32, tag="A")
                nc.vector.tensor_tensor_scan(
                    A_t[:], a_t[:], ones[:], 1.0, AL.mult, AL.mult
                )

                nc.scalar.copy(summA[:, m : m + 1], A_t[:, SC - 1 : SC])
                nc.scalar.copy(summH[:, m : m + 1], h_loc[:, SC - 1 : SC])

                GBUFS = 2 * CM
                gl = wk.tile([P, SC], MBF16, tag="gl", bufs=GBUFS, name=f"gl{m}")
                nc.vector.tensor_tensor(gl[:], h_loc[:], silu_t[:], AL.mult)
                gA = wk.tile([P, SC], MBF16, tag="gA", bufs=GBUFS, name=f"gA{m}")
                nc.vector.tensor_tensor(gA[:], A_t[:], silu_t[:], AL.mult)
                gl_tiles[m] = gl
                gA_tiles[m] = gA

                # ---- chunk end: issue summary AllGather (carry deferred) --------
                if (m + 1) % CM == 0 or m == MT - 1:
                    c = m // CM
                    m0 = c * CM
                    cm = m + 1 - m0
                    nc.gpsimd.dma_start(summ_d[c][:, 0:cm], summA[:, m0 : m0 + cm])
                    nc.gpsimd.dma_start(summ_d[c][:, CM : CM + cm],
                                        summH[:, m0 : m0 + cm])
                    nc.gpsimd.collective_compute(
                        "AllGather",
                        AL.bypass,
                        replica_groups=[list(range(NC))],
                        ins=[summ_d[c].opt()],
                        outs=[gath_d[c].opt()],
                    )
                    gsum = []
                    for r in range(NC):
                        gs = wk.tile([P, 2 * CM], F32, tag=f"gsum{r}",
                                     name=f"gsum{r}_{c}")
                        nc.gpsimd.dma_start(gs[:], gath_d[c][r])
                        gsum.append(gs)
                    pending.append((c, m0, cm, gsum))
            while pending:
                process_chunk(*pending.pop(0))
            scopeA.__exit__(None, None, None)

            # ---- phase D: down-projection out = g.T @ wo -----------------------
            scopeD = nc.named_scope("phaseD"); scopeD.__enter__()
            for nb in range(NB):
                ps_o = [pp.tile([P, 512], F32, tag="ps", name=f"pso{i}")
                        for i in range(MPT)]
                for m in range(MT):
                    g_rd = wk.tile([P, SC], MBF16, tag="g_rd", bufs=4)
                    nc.sync.dma_start(g_rd[:], g_d[m])
                    wo_rd = wk.tile([P, 512], MBF16, tag="wo_rd", bufs=4)
                    nc.sync.dma_start(wo_rd[:], wo_in[nb, m])
                    for mb in range(MPT):
                        nc.tensor.matmul(
                            ps_o[mb][:, :512],
                            g_rd[:, mb * P : (mb + 1) * P],
                            wo_rd[:],
                            start=(m == 0),
                            stop=(m == MT - 1),
                        )
                for mb in range(MPT):
                    o_sb = wk.tile([P, 512], F32, tag="o_sb", bufs=8)
                    if mb % 2 == 0:
                        nc.vector.tensor_copy(o_sb[:], ps_o[mb][:, :512])
                    else:
                        nc.scalar.copy(o_sb[:], ps_o[mb][:, :512])
                    # out writes ride the scalar hwdge queue so they never
                    # head-block the next pass's g/wo prefetch on sync; the
                    # final pass splits across both queues to shorten the tail.
                    dma_eng = nc.scalar if (nb < NB - 1 or mb % 2) else nc.sync
                    dma_eng.dma_start(
                        out_d[mb * P : (mb + 1) * P, nb * 512 : (nb + 1) * 512],
                        o_sb[:],
                    )
            scopeD.__exit__(None, None, None)
    nc.compile()
    return nc


def _prep_inputs(x, cu_seqlens, w_w, wz_w, wh_w, wo_w, conv_w, NC):
    """Host-side sharding + layout prep. Returns in_maps list."""
    S, D = x.shape[1], x.shape[2]
    H = w_w.shape[0]
    SC = S // NC
    KT, MT = D // P, H // P
    NB = D // 512

    xT = np.ascontiguousarray(x[0].T.astype(np.float32))  # (D, S)
    xt_full = np.zeros((D, S + HIST), np.float32)
    xt_full[:, HIST:] = xT

    start = np.zeros(S, np.float32)
    for v in np.asarray(cu_seqlens[:-1]):
        v = int(v)
        if 0 <= v < S:
            start[v] = 1.0
    u = 1.0 - start
    u_full = np.ones(S + 2, np.float32)
    u_full[2:] = u  # index t+2 <-> position t

    def wprep(wm):  # (H, D) -> (MT, P, KT, P) with [m,p,k,j] = w[m*P+j, k*P+p]
        return np.ascontiguousarray(
            wm.astype(np.float32).reshape(MT, P, KT, P).transpose(0, 3, 2, 1)
        ).astype(BF16)

    wz_t, wh_t, w_t = wprep(wz_w), wprep(wh_w), wprep(w_w)
    # wo: [NB, MT, P, 512] with [nb,m,p,j] = wo[nb*512+j, m*128+p]
    wo_t = np.ascontiguousarray(
        wo_w.T.astype(np.float32).reshape(MT, P, NB, 512).transpose(2, 0, 1, 3)
    ).astype(BF16)
    cw_t = np.ascontiguousarray(conv_w.astype(np.float32).reshape(MT, P, CONV))

    in_maps = []
    for c in range(NC):
        s0 = c * SC
        xt_c = np.ascontiguousarray(
            xt_full[:, s0 : s0 + SC + HIST]
            .reshape(KT, P, SC + HIST)
            .transpose(1, 0, 2)
        ).astype(BF16)
        u_c = np.ascontiguousarray(
            np.broadcast_to(u_full[s0 : s0 + SC + 2], (P, SC + 2))
        ).astype(BF16)
        sel_c = np.zeros((P, NC), np.float32)
        sel_c[:, c] = 1.0
        in_maps.append(
            {
                "xt": xt_c,
                "wz": wz_t,
                "wh": wh_t,
                "w": w_t,
                "wo": wo_t,
                "cw": cw_t,
                "u": u_c,
                "sel": sel_c,
            }
        )
    return in_maps


_NC_CACHE = {}


def run_gru(x, cu_seqlens, w_w, wz_w, wh_w, wo_w, conv_w, NC=8, trace=False):
    S, D = x.shape[1], x.shape[2]
    H = w_w.shape[0]
    SC = S // NC
    key = (D, H, SC, NC)
    if key not in _NC_CACHE:
        _NC_CACHE[key] = build_gru_kernel(D, H, SC, NC)
    nc = _NC_CACHE[key]
    in_maps = _prep_inputs(x, cu_seqlens, w_w, wz_w, wh_w, wo_w, conv_w, NC)
    res = run_bass_kernel_spmd(nc, in_maps, list(range(NC)), trace=trace)
    out = np.concatenate([res.results[c]["out"] for c in range(NC)], axis=0)
    return out.reshape(1, S, D).astype(np.float32), res


def kernel(**inputs):
    out, _ = run_gru(
        inputs["x"],
        inputs["cu_seqlens"],
        inputs["w_w"],
        inputs["wz_w"],
        inputs["wh_w"],
        inputs["wo_w"],
        inputs["conv_w"],
        NC=8,
    )
    return out


# revision 25
# speedup vs baseline: 1.0187x; 1.0132x over previous
"""Trainium2 Bass kernel for nn_GRU_15461882266204 (minGRU with causal conv gate).

Math (reference):
  w0 = x @ w_w.T ; z0 = x @ wz_w.T ; th = x @ wh_w.T          (S,H)
  z  = sigmoid(causal_conv4(z0, conv_w, segment-masked))
  a  = (1-z) * (1-start) ; b = z * th
  h_t = a_t * h_{t-1} + b_t                                    (scan over S)
  out = (h * silu(w0)) @ wo_w.T                                (S,D)

Strategy (v2): sequence-parallel over 8 NeuronCores (1024 positions each, all
5632 channels per core).  One fused phase A computes, per 128-channel m-tile:
the z / th / w0 projections (k-outer matmul groups sharing the xt moving
operand), the masked causal conv + sigmoid gates, the two hardware
tensor_tensor_scans (h_local, cumprod A), silu, and the bf16 products
g_loc = h_local*silu, gA = A*silu which stay SBUF-resident.  Cross-core scan
carry is exchanged in 11 chunked AllGathers of (A_end, h_end) summaries (4
m-tiles each) issued on the gpsimd queue as soon as each chunk's scans finish,
so collective latency overlaps the next chunk's matmuls.  The carry fixup
g = g_loc + carry*gA lands during phase A and writes final bf16 g tiles to
DRAM.  Phase D then runs 4 column passes of down-projection matmuls reading
pre-fixed g (2KB rows) + per-pass wo slices, sequence-sharded output (host
concatenates; no all-reduce).
"""
import sys

sys.path.insert(0, "/opt/trn_rl_repo")

import numpy as np

import concourse.bacc as bacc
import concourse.mybir as mybir
import concourse.tile as tile
from concourse.bass_utils import run_bass_kernel_spmd

try:
    import ml_dtypes

    BF16 = np.dtype(ml_dtypes.bfloat16)
except ImportError:  # pragma: no cover
    BF16 = None

F32 = mybir.dt.float32
MBF16 = mybir.dt.bfloat16
AL = mybir.AluOpType
ACTF = mybir.ActivationFunctionType

P = 128
CONV = 4
# 3 history columns are needed for the conv taps; pad to 4 (one dead leading
# column) to keep everything 4-aligned.
HIST = 4
CM = 4  # m-tiles per carry-exchange chunk


def build_gru_kernel(D, H, SC, NC):
    """Build the SPMD per-core program. SC = sequence chunk per core."""
    KT = D // P    # contraction k-tiles
    MT = H // P    # hidden m-tiles
    SCH = SC + HIST
    MPT = SC // P              # output row tiles (s on partitions)
    NB = D // 512              # down-proj column passes
    NCH = (MT + CM - 1) // CM  # carry chunks

    nc = bacc.Bacc(None, target_bir_lowering=False, debug=False)

    xt_in = nc.declare_dram_parameter("xt", [P, KT, SCH], MBF16, isOutput=False)
    wz_in = nc.declare_dram_parameter("wz", [MT, P, KT, P], MBF16, isOutput=False)
    wh_in = nc.declare_dram_parameter("wh", [MT, P, KT, P], MBF16, isOutput=False)
    w_in = nc.declare_dram_parameter("w", [MT, P, KT, P], MBF16, isOutput=False)
    wo_in = nc.declare_dram_parameter("wo", [NB, MT, P, 512], MBF16, isOutput=False)
    cw_in = nc.declare_dram_parameter("cw", [MT, P, CONV], F32, isOutput=False)
    u_in = nc.declare_dram_parameter("u", [P, SC + 2], MBF16, isOutput=False)
    sel_in = nc.declare_dram_parameter("sel", [P, NC], F32, isOutput=False)
    # output in bf16: halves the un-overlappable final out-DMA; host upcasts
    out_d = nc.declare_dram_parameter("out", [SC, D], MBF16, isOutput=True)

    with tile.TileContext(nc) as tc:
        with (
            tc.tile_pool(name="const", bufs=1) as cpool,
            tc.tile_pool(name="wts", bufs=3) as wpool,
            tc.tile_pool(name="work", bufs=2) as wk,
            tc.tile_pool(name="psum", bufs=8, space="PSUM") as pp,
            tc.tile_pool(name="dram", bufs=1, space="DRAM") as dp,
        ):
            # ---- resident tiles -------------------------------------------------
            # Ramp-critical ordering: the very first matmul needs wz[0] and xt
            # chunk 0 — issue those DMAs first so the fair-share DMA engines
            # complete them before the bulk (xt tail, u, sel) transfers.
            XCH = 8
            wz0_sb = wpool.tile([P, KT, P], MBF16, tag="wz", name="wz0_sb")
            nc.sync.dma_start(wz0_sb[:, 0:KT // 2, :], wz_in[0, :, 0:KT // 2, :])
            nc.sync.dma_start(wz0_sb[:, KT // 2:, :], wz_in[0, :, KT // 2:, :])
            xt_sb = []
            xj = cpool.tile([P, KT // XCH, SCH], MBF16, tag="xt0", name="xt0")
            nc.sync.dma_start(xj[:], xt_in[:, 0:KT // XCH, :])
            xt_sb.append(xj)
            wh0_sb = wpool.tile([P, KT, P], MBF16, tag="wh", name="wh0_sb")
            nc.sync.dma_start(wh0_sb[:], wh_in[0])
            for j in range(1, XCH):
                xj = cpool.tile([P, KT // XCH, SCH], MBF16, tag=f"xt{j}",
                                name=f"xt{j}")
                nc.sync.dma_start(xj[:], xt_in[:, j * (KT // XCH):(j + 1) * (KT // XCH), :])
                xt_sb.append(xj)
            w0_sb = wpool.tile([P, KT, P], MBF16, tag="w", name="w0_sb")
            nc.sync.dma_start(w0_sb[:], w_in[0])
            cw0_sb = wk.tile([P, CONV], F32, tag="cw", name="cw0_sb")
            nc.sync.dma_start(cw0_sb[:], cw_in[0])

            def xt_slice(k, lo, hi):
                j = k // (KT // XCH)
                return xt_sb[j][:, k % (KT // XCH), lo:hi]

            u_sb = cpool.tile([P, SC + 2], MBF16, tag="u")
            nc.sync.dma_start(u_sb[:], u_in[:])
            sel_sb = cpool.tile([P, NC], F32, tag="sel")
            nc.sync.dma_start(sel_sb[:], sel_in[:])
            ones = cpool.tile([P, SC], MBF16, tag="ones")
            nc.any.memset(ones[:], 1.0)
            summA = cpool.tile([P, MT], F32, tag="summA")
            summH = cpool.tile([P, MT], F32, tag="summH")
            carry = cpool.tile([P, MT], F32, tag="carry")
            nc.vector.memset(carry[:], 0.0)

            # internal DRAM buffers
            g_d = dp.tile([MT, P, SC], MBF16)
            summ_d = [dp.tile([P, 2 * CM], F32, tag=f"summ{c}", name=f"summ{c}")
                      for c in range(NCH)]
            gath_d = [dp.tile([NC, P, 2 * CM], F32, addr_space="Shared",
                              tag=f"gath{c}", name=f"gath{c}")
                      for c in range(NCH)]

            # Warmup collective: absorb the first-rendezvous latency across
            # cores while the first m-tiles compute.
            warm_d = dp.tile([P, 2 * CM], F32, tag="warm", name="warm_d")
            warmg_d = dp.tile([NC, P, 2 * CM], F32, addr_space="Shared",
                              tag="warmg", name="warmg_d")
            warm_sb = cpool.tile([P, 2 * CM], F32, tag="warm_sb")
            nc.vector.memset(warm_sb[:], 0.0)
            nc.gpsimd.dma_start(warm_d[:], warm_sb[:])
            nc.gpsimd.collective_compute(
                "AllGather",
                AL.bypass,
                replica_groups=[list(range(NC))],
                ins=[warm_d.opt()],
                outs=[warmg_d.opt()],
            )

            gl_tiles = {}
            gA_tiles = {}
            pending = []  # chunks whose carry chain/fixup is deferred

            def process_chunk(c, m0, cm, gsum):
                """Emit carry chain + fixup for chunk c (gath already read)."""
                state = wk.tile([P, CM], F32, tag="cstate", name=f"cst{c}")
                tmp_c = wk.tile([P, CM], F32, tag="ctmp", name=f"ctm{c}")
                nc.vector.memset(state[:], 0.0)
                for r in range(NC):
                    if r > 0:
                        nc.vector.scalar_tensor_tensor(
                            carry[:, m0 : m0 + cm], state[:, :cm],
                            sel_sb[:, r : r + 1], carry[:, m0 : m0 + cm],
                            AL.mult, AL.add,
                        )
                    if r < NC - 1:
                        nc.vector.tensor_tensor(
                            tmp_c[:, :cm], state[:, :cm], gsum[r][:, 0:cm],
                            AL.mult
                        )
                        nc.vector.tensor_tensor(
                            state[:, :cm], tmp_c[:, :cm],
                            gsum[r][:, CM : CM + cm], AL.add
                        )
                for mm in range(m0, m0 + cm):
                    gfix = wk.tile([P, SC], MBF16, tag="gfix", name=f"gfx{mm}")
                    nc.vector.scalar_tensor_tensor(
                        gfix[:], gA_tiles[mm][:], carry[:, mm : mm + 1],
                        gl_tiles[mm][:], AL.mult, AL.add,
                    )
                    nc.gpsimd.dma_start(g_d[mm], gfix[:])
                    del gl_tiles[mm], gA_tiles[mm]

            scopeA = nc.named_scope("phaseA"); scopeA.__enter__()
            for m in range(MT):
                # flush deferred carry chunks once the collective has had ~2
                # m-tiles (~50us) to complete — keeps the DVE FIFO from
                # head-blocking on the gather.
                while pending and m >= pending[0][1] + pending[0][2] + 2:
                    process_chunk(*pending.pop(0))
                if m == 0:
                    cw_sb, wz_sb, wh_sb, w_sb = cw0_sb, wz0_sb, wh0_sb, w0_sb
                else:
                    cw_sb = wk.tile([P, CONV], F32, tag="cw")
                    nc.sync.dma_start(cw_sb[:], cw_in[m])
                    wz_sb = wpool.tile([P, KT, P], MBF16, tag="wz")
                    nc.sync.dma_start(wz_sb[:], wz_in[m])
                    wh_sb = wpool.tile([P, KT, P], MBF16, tag="wh")
                    nc.sync.dma_start(wh_sb[:], wh_in[m])
                    w_sb = wpool.tile([P, KT, P], MBF16, tag="w")
                    nc.sync.dma_start(w_sb[:], w_in[m])

                # z_pre = wz_m.T @ x over SC+4 cols (history included), k-outer
                ps_z = [pp.tile([P, 512], F32, tag="ps", name=f"psz{i}")
                        for i in range(3)]
                for k in range(KT):
                    nc.tensor.matmul(ps_z[0][:, :512], wz_sb[:, k, :],
                                     xt_slice(k, 0, 512),
                                     start=(k == 0), stop=(k == KT - 1))
                    nc.tensor.matmul(ps_z[1][:, :512], wz_sb[:, k, :],
                                     xt_slice(k, 512, 1024),
                                     start=(k == 0), stop=(k == KT - 1))
                    nc.tensor.matmul(ps_z[2][:, :SCH - 1024], wz_sb[:, k, :],
                                     xt_slice(k, 1024, SCH),
                                     start=(k == 0), stop=(k == KT - 1))
                z_pre = wk.tile([P, SCH], MBF16, tag="zpre")
                nc.scalar.copy(z_pre[:, 0:512], ps_z[0][:, :512])
                nc.scalar.copy(z_pre[:, 512:1024], ps_z[1][:, :512])
                nc.scalar.copy(z_pre[:, 1024:SCH], ps_z[2][:, :SCH - 1024])

                # th matmuls (positions [0, SC) = cols [HIST, SCH))
                ps_h = [pp.tile([P, 512], F32, tag="ps", name=f"psh{i}")
                        for i in range(2)]
                for k in range(KT):
                    for i in range(2):
                        nc.tensor.matmul(
                            ps_h[i][:, :512], wh_sb[:, k, :],
                            xt_slice(k, HIST + i * 512, HIST + (i + 1) * 512),
                            start=(k == 0), stop=(k == KT - 1))
                th_sb = wk.tile([P, SC], MBF16, tag="th", bufs=3)
                nc.scalar.copy(th_sb[:, 0:512], ps_h[0][:, :512])
                nc.scalar.copy(th_sb[:, 512:1024], ps_h[1][:, :512])

                # w0 matmuls + silu
                ps_w = [pp.tile([P, 512], F32, tag="ps", name=f"psw{i}")
                        for i in range(2)]
                for k in range(KT):
                    for i in range(2):
                        nc.tensor.matmul(
                            ps_w[i][:, :512], w_sb[:, k, :],
                            xt_slice(k, HIST + i * 512, HIST + (i + 1) * 512),
                            start=(k == 0), stop=(k == KT - 1))
                silu_t = wk.tile([P, SC], MBF16, tag="silu")
                nc.scalar.activation(silu_t[:, 0:512], ps_w[0][:, :512], ACTF.Silu)
                nc.scalar.activation(silu_t[:, 512:1024], ps_w[1][:, :512], ACTF.Silu)

                # masked shifted taps: yk(t) = u(t) * y{k-1}(t-1), y0 = z_pre
                y1 = wk.tile([P, SC + 2], MBF16, tag="y1")
                nc.vector.tensor_tensor(
                    y1[:], u_sb[:, : SC + 2], z_pre[:, HIST - 3 : HIST - 3 + SC + 2],
                    AL.mult,
                )
                y2 = wk.tile([P, SC + 1], MBF16, tag="y2")
                nc.vector.tensor_tensor(
                    y2[:], u_sb[:, 1 : SC + 2], y1[:, : SC + 1], AL.mult
                )
                y3 = wk.tile([P, SC], MBF16, tag="y3")
                nc.vector.tensor_tensor(
                    y3[:], u_sb[:, 2 : SC + 2], y2[:, :SC], AL.mult
                )
                # conv accumulation: acc = z*cw3 + y1*cw2 + y2*cw1 + y3*cw0
                acc = wk.tile([P, SC], F32, tag="acc")
                nc.vector.tensor_scalar(
                    acc[:], z_pre[:, HIST:SCH], cw_sb[:, 3:4], None, AL.mult
                )
                nc.vector.scalar_tensor_tensor(
                    acc[:], y1[:, 2 : SC + 2], cw_sb[:, 2:3], acc[:], AL.mult, AL.add
                )
                nc.vector.scalar_tensor_tensor(
                    acc[:], y2[:, 1 : SC + 1], cw_sb[:, 1:2], acc[:], AL.mult, AL.add
                )
                nc.vector.scalar_tensor_tensor(
                    acc[:], y3[:, :SC], cw_sb[:, 0:1], acc[:], AL.mult, AL.add
                )

                z_t = wk.tile([P, SC], MBF16, tag="zt")
                nc.scalar.activation(z_t[:], acc[:], ACTF.Sigmoid)
                na = wk.tile([P, SC], MBF16, tag="na")
                nc.scalar.activation(na[:], acc[:], ACTF.Sigmoid, scale=-1.0)

                a_t = wk.tile([P, SC], MBF16, tag="a")
                nc.vector.tensor_tensor(a_t[:], na[:], u_sb[:, 2 : SC + 2], AL.mult)
                b_t = wk.tile([P, SC], MBF16, tag="b")
                nc.vector.tensor_tensor(b_t[:], z_t[:], th_sb[:], AL.mult)

                h_loc = wk.tile([P, SC], F32, tag="hl")
                nc.vector.tensor_tensor_scan(
                    h_loc[:], a_t[:], b_t[:], 0.0, AL.mult, AL.add
                )
                A_t = wk.tile([P, SC], F32, tag="A")
                nc.vector.tensor_tensor_scan(
                    A_t[:], a_t[:], ones[:], 1.0, AL.mult, AL.mult
                )

                nc.scalar.copy(summA[:, m : m + 1], A_t[:, SC - 1 : SC])
                nc.scalar.copy(summH[:, m : m + 1], h_loc[:, SC - 1 : SC])

                GBUFS = 2 * CM
                gl = wk.tile([P, SC], MBF16, tag="gl", bufs=GBUFS, name=f"gl{m}")
                nc.vector.tensor_tensor(gl[:], h_loc[:], silu_t[:], AL.mult)
                gA = wk.tile([P, SC], MBF16, tag="gA", bufs=GBUFS, name=f"gA{m}")
                nc.vector.tensor_tensor(gA[:], A_t[:], silu_t[:], AL.mult)
                gl_tiles[m] = gl
                gA_tiles[m] = gA

                # ---- chunk end: issue summary AllGather (carry deferred) --------
                if (m + 1) % CM == 0 or m == MT - 1:
                    c = m // CM
                    m0 = c * CM
                    cm = m + 1 - m0
                    nc.gpsimd.dma_start(summ_d[c][:, 0:cm], summA[:, m0 : m0 + cm])
                    nc.gpsimd.dma_start(summ_d[c][:, CM : CM + cm],
                                        summH[:, m0 : m0 + cm])
                    nc.gpsimd.collective_compute(
                        "AllGather",
                        AL.bypass,
                        replica_groups=[list(range(NC))],
                        ins=[summ_d[c].opt()],
                        outs=[gath_d[c].opt()],
                    )
                    gsum = []
                    for r in range(NC):
                        gs = wk.tile([P, 2 * CM], F32, tag=f"gsum{r}",
                                     name=f"gsum{r}_{c}")
                        nc.gpsimd.dma_start(gs[:], gath_d[c][r])
                        gsum.append(gs)
                    pending.append((c, m0, cm, gsum))
            while pending:
                process_chunk(*pending.pop(0))
            scopeA.__exit__(None, None, None)

            # ---- phase D: down-projection out = g.T @ wo -----------------------
            scopeD = nc.named_scope("phaseD"); scopeD.__enter__()
            for nb in range(NB):
                ps_o = [pp.tile([P, 512], F32, tag="ps", name=f"pso{i}")
                        for i in range(MPT)]
                for m in range(MT):
                    g_rd = wk.tile([P, SC], MBF16, tag="g_rd", bufs=6)
                    nc.sync.dma_start(g_rd[:], g_d[m])
                    wo_rd = wk.tile([P, 512], MBF16, tag="wo_rd", bufs=6)
                    nc.sync.dma_start(wo_rd[:], wo_in[nb, m])
                    for mb in range(MPT):
                        nc.tensor.matmul(
                            ps_o[mb][:, :512],
                            g_rd[:, mb * P : (mb + 1) * P],
                            wo_rd[:],
                            start=(m == 0),
                            stop=(m == MT - 1),
                        )
                for mb in range(MPT):
                    o_sb = wk.tile([P, 512], MBF16, tag="o_sb", bufs=8)
                    if mb % 2 == 0:
                        nc.vector.tensor_copy(o_sb[:], ps_o[mb][:, :512])
                    else:
                        nc.scalar.copy(o_sb[:], ps_o[mb][:, :512])
                    # out writes ride the scalar hwdge queue so they never
                    # head-block the next pass's g/wo prefetch on sync; the
                    # final pass splits across both queues to shorten the tail.
                    dma_eng = nc.scalar if (nb < NB - 1 or mb % 2) else nc.sync
                    dma_eng.dma_start(
                        out_d[mb * P : (mb + 1) * P, nb * 512 : (nb + 1) * 512],
                        o_sb[:],
                    )
            scopeD.__exit__(None, None, None)
    nc.compile()
    return nc


def _prep_inputs(x, cu_seqlens, w_w, wz_w, wh_w, wo_w, conv_w, NC):
    """Host-side sharding + layout prep. Returns in_maps list."""
    S, D = x.shape[1], x.shape[2]
    H = w_w.shape[0]
    SC = S // NC
    KT, MT = D // P, H // P
    NB = D // 512

    xT = np.ascontiguousarray(x[0].T.astype(np.float32))  # (D, S)
    xt_full = np.zeros((D, S + HIST), np.float32)
    xt_full[:, HIST:] = xT

    start = np.zeros(S, np.float32)
    for v in np.asarray(cu_seqlens[:-1]):
        v = int(v)
        if 0 <= v < S:
            start[v] = 1.0
    u = 1.0 - start
    u_full = np.ones(S + 2, np.float32)
    u_full[2:] = u  # index t+2 <-> position t

    def wprep(wm):  # (H, D) -> (MT, P, KT, P) with [m,p,k,j] = w[m*P+j, k*P+p]
        return np.ascontiguousarray(
            wm.astype(np.float32).reshape(MT, P, KT, P).transpose(0, 3, 2, 1)
        ).astype(BF16)

    wz_t, wh_t, w_t = wprep(wz_w), wprep(wh_w), wprep(w_w)
    # wo: [NB, MT, P, 512] with [nb,m,p,j] = wo[nb*512+j, m*128+p]
    wo_t = np.ascontiguousarray(
        wo_w.T.astype(np.float32).reshape(MT, P, NB, 512).transpose(2, 0, 1, 3)
    ).astype(BF16)
    cw_t = np.ascontiguousarray(conv_w.astype(np.float32).reshape(MT, P, CONV))

    in_maps = []
    for c in range(NC):
        s0 = c * SC
        xt_c = np.ascontiguousarray(
            xt_full[:, s0 : s0 + SC + HIST]
            .reshape(KT, P, SC + HIST)
            .transpose(1, 0, 2)
        ).astype(BF16)
        u_c = np.ascontiguousarray(
            np.broadcast_to(u_full[s0 : s0 + SC + 2], (P, SC + 2))
        ).astype(BF16)
        sel_c = np.zeros((P, NC), np.float32)
        sel_c[:, c] = 1.0
        in_maps.append(
            {
                "xt": xt_c,
                "wz": wz_t,
                "wh": wh_t,
                "w": w_t,
                "wo": wo_t,
                "cw": cw_t,
                "u": u_c,
                "sel": sel_c,
            }
        )
    return in_maps


_NC_CACHE = {}


def run_gru(x, cu_seqlens, w_w, wz_w, wh_w, wo_w, conv_w, NC=8, trace=False):
    S, D = x.shape[1], x.shape[2]
    H = w_w.shape[0]
    SC = S // NC
    key = (D, H, SC, NC)
    if key not in _NC_CACHE:
        _NC_CACHE[key] = build_gru_kernel(D, H, SC, NC)
    nc = _NC_CACHE[key]
    in_maps = _prep_inputs(x, cu_seqlens, w_w, wz_w, wh_w, wo_w, conv_w, NC)
    res = run_bass_kernel_spmd(nc, in_maps, list(range(NC)), trace=trace)
    out = np.concatenate([res.results[c]["out"] for c in range(NC)], axis=0)
    return out.reshape(1, S, D).astype(np.float32), res


def kernel(**inputs):
    out, _ = run_gru(
        inputs["x"],
        inputs["cu_seqlens"],
        inputs["w_w"],
        inputs["wz_w"],
        inputs["wh_w"],
        inputs["wo_w"],
        inputs["conv_w"],
        NC=8,
    )
    return out
